# revision 15
# baseline (speedup 1.0000x reference)
"""AdversarialMorphingLoss — Trainium2 Bass kernel (8-core data parallel).

Full inputs arrive on the host; we shard the batch dim (B=4096) into 8
contiguous blocks of 512 rows, run one SPMD Bass program on all 8
NeuronCores, and each core returns the partial (un-normalized) sum of the
per-sample loss contribution over its 512 rows.  The host sums the 8
partials and divides by B.

Per-sample math (matching reference.py):
  scores_b = 100/S * sum_s inc_s * CONFIG_MULT[pid % 4]
  inc_s    = 0.6*(sz_s > 1400) + 0.4*(dly_s < 0.05)
           + 0.2*(|sz_s - sz_{s-1}| < 0.5) + 0.1*(dir_s != dir_{s-1})
  with sz[:, -1] -> min(sz[:, -1] + pad*1500, 1500), dly[:, -1] += delay_ms,
  and the s=0 "prev" being -1.0 (so the dir term contributes 0.1 at s=0 and
  the size-equality term contributes 0).

  c_b = (2/30)*relu(scores-15) + 0.5*(|dly_ms - TD[pid]| + |pad - TP[pid]|)
      + 0.3*(relu(dly_ms-20)/20 + relu(pad-0.3)) + 0.2*(conf - (scores<30))^2
  loss = mean_b c_b

On-device strategy (memory-bound: streams 96 MB of traces):
  * count (sz > 1400) over all S int32 cols with one ScalarE
    activation(Sign, bias=-1400.5, accum_out=...) per [128, 2048] tile
    (integers never hit the .5 threshold -> exact), then patch the last
    (float-modified) column with exact [128,4] is_gt ops.
  * count (dly < 0.05) the same way via Sign(0.05 - dly).
  * count consecutive-size equality / direction flips with one fused
    VectorE tensor_tensor_reduce(is_equal / not_equal, accum_out=...) per
    tile, again patching the last column separately.
  * everything per-sample afterwards runs on tiny [128, 4] tiles.
"""

import numpy as np
from contextlib import ExitStack

import concourse.bass as bass
import concourse.bacc as bacc
import concourse.mybir as mybir
from concourse import tile
from concourse.bass_utils import run_bass_kernel_spmd

B, S = 4096, 2048
N_CORES = 8
BC = B // N_CORES          # 512 rows per core
P = 128                    # SBUF partitions
NT = BC // P               # 4 tiles of 128 rows per core

F32 = mybir.dt.float32
I32 = mybir.dt.int32
ALU = mybir.AluOpType
ACTF = mybir.ActivationFunctionType

_NC_CACHE = None
LAST_RESULTS = None        # BassKernelResults of the last kernel() call


def _build_nc() -> bass.Bass:
    nc = bacc.Bacc()

    sz_h = nc.declare_dram_parameter("raw_sizes", [BC, S], I32, isOutput=False)
    dl_h = nc.declare_dram_parameter("raw_delays", [BC, S], F32, isOutput=False)
    dr_h = nc.declare_dram_parameter("raw_directions", [BC, S], I32, isOutput=False)
    dms_h = nc.declare_dram_parameter("delay_ms", [BC], F32, isOutput=False)
    pad_h = nc.declare_dram_parameter("padding_norm", [BC], F32, isOutput=False)
    cnf_h = nc.declare_dram_parameter("confidence", [BC], F32, isOutput=False)
    pid_h = nc.declare_dram_parameter("profile_ids", [BC], I32, isOutput=False)
    out_h = nc.declare_dram_parameter("partial", [P, 1], F32, isOutput=True)

    with tile.TileContext(nc) as tc, ExitStack() as ctx:
        inp = ctx.enter_context(tc.tile_pool(name="inp", bufs=3))
        scr = ctx.enter_context(tc.tile_pool(name="scr", bufs=2))
        sm = ctx.enter_context(tc.tile_pool(name="sm", bufs=1))

        def smt(tag, dtype=F32):
            return sm.tile([P, NT], dtype, tag=tag, name=tag)

        _consts = {}

        def constv(val):
            """[128,1] f32 SBUF tile holding `val` (for activation bias APs)."""
            if val not in _consts:
                cname = f"cst{len(_consts)}"
                ct = sm.tile([P, 1], F32, tag=cname, name=cname)
                nc.vector.memset(ct[:, :], val)
                _consts[val] = ct[:, :]
            return _consts[val]

        # Per-row (per-sample) vectors, laid out [partition, tile]: (p, t) = v[t*128 + p]
        dvec = smt("dvec")
        pvec = smt("pvec")
        cvec = smt("cvec")
        pidt = smt("pidt", I32)
        nc.sync.dma_start(dvec[:, :], dms_h[:].rearrange("(t p) -> p t", p=P))
        nc.sync.dma_start(pvec[:, :], pad_h[:].rearrange("(t p) -> p t", p=P))
        nc.sync.dma_start(cvec[:, :], cnf_h[:].rearrange("(t p) -> p t", p=P))
        nc.sync.dma_start(pidt[:, :], pid_h[:].rearrange("(t p) -> p t", p=P))

        # Big-op accumulators (per tile column)
        R1 = smt("R1")     # sum sign(sz - 1400.5)           over all S cols
        R2 = smt("R2")     # sum sign(0.05 - dly)            over all S cols
        R3 = smt("R3")     # count sz[s] == sz[s-1],  s=1..S-1 (unmodified last col)
        R4 = smt("R4")     # count dir[s] != dir[s-1], s=1..S-1
        szlast = smt("szlast")   # f32 copy of int sz[:, S-1]
        szprev = smt("szprev")   # f32 copy of int sz[:, S-2]
        dllast = smt("dllast")   # f32 copy of dly[:, S-1]

        for t in range(NT):
            rows = slice(t * P, (t + 1) * P)
            szt = inp.tile([P, S], I32, tag="szt")
            dlt = inp.tile([P, S], F32, tag="dlt")
            drt = inp.tile([P, S], I32, tag="drt")
            nc.sync.dma_start(szt[:, :], sz_h[rows, :])
            nc.sync.dma_start(dlt[:, :], dl_h[rows, :])
            nc.sync.dma_start(drt[:, :], dr_h[rows, :])

            col = slice(t, t + 1)
            o1 = scr.tile([P, S], F32, tag="o1")
            nc.scalar.activation(o1[:, :], szt[:, :], ACTF.Sign,
                                 bias=constv(-1400.5), scale=1.0, accum_out=R1[:, col])
            o2 = scr.tile([P, S], F32, tag="o2")
            nc.scalar.activation(o2[:, :], dlt[:, :], ACTF.Sign,
                                 bias=constv(0.05), scale=-1.0, accum_out=R2[:, col])
            # fused compare + row-sum on DVE: out = (in0 bypass 0) cmp in1,
            # accum_out = sum(out).  (tensor_tensor_reduce crashes the HW
            # runtime in this toolchain; scalar_tensor_tensor w/ accum works.)
            o3 = scr.tile([P, S - 1], F32, tag="o3")
            nc.vector.scalar_tensor_tensor(
                o3[:, :], szt[:, 1:S], 0.0, szt[:, 0:S - 1],
                ALU.bypass, ALU.is_equal, accum_out=R3[:, col])
            o4 = scr.tile([P, S - 1], F32, tag="o4")
            nc.vector.scalar_tensor_tensor(
                o4[:, :], drt[:, 1:S], 0.0, drt[:, 0:S - 1],
                ALU.bypass, ALU.not_equal, accum_out=R4[:, col])

            nc.vector.tensor_copy(szlast[:, col], szt[:, S - 1:S])
            nc.vector.tensor_copy(szprev[:, col], szt[:, S - 2:S - 1])
            nc.vector.tensor_copy(dllast[:, col], dlt[:, S - 1:S])

        # ---- per-sample combine, all on [128, 4] tiles (VectorE only,
        # to keep per-instruction sync-wait counts low on ScalarE) ----
        v = nc.vector

        # profile-id one-hots (pid in 0..4)
        pidf = smt("pidf")
        v.tensor_copy(pidf[:, :], pidt[:, :])
        e1, e2, e3, e4 = smt("e1"), smt("e2"), smt("e3"), smt("e4")
        v.tensor_scalar(e1[:, :], pidf[:, :], 1.0, None, ALU.is_equal)
        v.tensor_scalar(e2[:, :], pidf[:, :], 2.0, None, ALU.is_equal)
        v.tensor_scalar(e3[:, :], pidf[:, :], 3.0, None, ALU.is_equal)
        v.tensor_scalar(e4[:, :], pidf[:, :], 4.0, None, ALU.is_equal)

        # CONFIG_MULT[pid % 4] = 1.0 + 0.3*e1 + 0.6*e2 + 1.0*e3  (pid=4 -> 1.0)
        mlt = smt("mlt")
        v.tensor_scalar(mlt[:, :], e1[:, :], 0.3, 1.0, ALU.mult, ALU.add)
        v.scalar_tensor_tensor(mlt[:, :], e2[:, :], 0.6, mlt[:, :], ALU.mult, ALU.add)
        v.tensor_add(mlt[:, :], mlt[:, :], e3[:, :])

        # TARGET_DELAY[pid] = 2 - 1*e1 - 1.5*e2 + 3*e3 + 1*e4
        td = smt("td")
        v.tensor_scalar(td[:, :], e1[:, :], -1.0, 2.0, ALU.mult, ALU.add)
        v.scalar_tensor_tensor(td[:, :], e2[:, :], -1.5, td[:, :], ALU.mult, ALU.add)
        v.scalar_tensor_tensor(td[:, :], e3[:, :], 3.0, td[:, :], ALU.mult, ALU.add)
        v.tensor_add(td[:, :], td[:, :], e4[:, :])

        # TARGET_PAD[pid] = 0.08 + 0.04*e1 - 0.03*e2 + 0.07*e3 + 0.02*e4
        tp = smt("tp")
        v.tensor_scalar(tp[:, :], e1[:, :], 0.04, 0.08, ALU.mult, ALU.add)
        v.scalar_tensor_tensor(tp[:, :], e2[:, :], -0.03, tp[:, :], ALU.mult, ALU.add)
        v.scalar_tensor_tensor(tp[:, :], e3[:, :], 0.07, tp[:, :], ALU.mult, ALU.add)
        v.scalar_tensor_tensor(tp[:, :], e4[:, :], 0.02, tp[:, :], ALU.mult, ALU.add)

        # last-column morphing fixups
        padx = smt("padx")
        v.tensor_scalar(padx[:, :], pvec[:, :], 1500.0, None, ALU.mult)
        szmod = smt("szmod")
        v.tensor_add(szmod[:, :], szlast[:, :], padx[:, :])
        v.tensor_scalar(szmod[:, :], szmod[:, :], 1500.0, None, ALU.min)
        dlmod = smt("dlmod")
        v.tensor_add(dlmod[:, :], dllast[:, :], dvec[:, :])

        g1m, g1r = smt("g1m"), smt("g1r")
        v.tensor_scalar(g1m[:, :], szmod[:, :], 1400.0, None, ALU.is_gt)
        v.tensor_scalar(g1r[:, :], szlast[:, :], 1400.0, None, ALU.is_gt)
        l2m, l2r = smt("l2m"), smt("l2r")
        v.tensor_scalar(l2m[:, :], dlmod[:, :], 0.05, None, ALU.is_lt)
        v.tensor_scalar(l2r[:, :], dllast[:, :], 0.05, None, ALU.is_lt)
        e3r = smt("e3r")
        v.tensor_tensor(e3r[:, :], szlast[:, :], szprev[:, :], ALU.is_equal)
        d3 = smt("d3")
        v.tensor_sub(d3[:, :], szmod[:, :], szprev[:, :])
        a3 = smt("a3")
        nc.scalar.activation(a3[:, :], d3[:, :], ACTF.Abs)
        e3m = smt("e3m")
        v.tensor_scalar(e3m[:, :], a3[:, :], 0.5, None, ALU.is_lt)

        # exact per-row counts
        cnt1 = smt("cnt1")
        v.tensor_scalar(cnt1[:, :], R1[:, :], 0.5, float(S) / 2, ALU.mult, ALU.add)
        v.tensor_sub(cnt1[:, :], cnt1[:, :], g1r[:, :])
        v.tensor_add(cnt1[:, :], cnt1[:, :], g1m[:, :])
        cnt2 = smt("cnt2")
        v.tensor_scalar(cnt2[:, :], R2[:, :], 0.5, float(S) / 2, ALU.mult, ALU.add)
        v.tensor_sub(cnt2[:, :], cnt2[:, :], l2r[:, :])
        v.tensor_add(cnt2[:, :], cnt2[:, :], l2m[:, :])
        cnt3 = smt("cnt3")
        v.tensor_sub(cnt3[:, :], R3[:, :], e3r[:, :])
        v.tensor_add(cnt3[:, :], cnt3[:, :], e3m[:, :])

        # scores = (0.6*c1 + 0.4*c2 + 0.2*c3 + 0.1*c4 + 0.1) * (100/S) * mult
        acc = smt("acc")
        v.tensor_scalar(acc[:, :], cnt1[:, :], 0.6, None, ALU.mult)
        v.scalar_tensor_tensor(acc[:, :], cnt2[:, :], 0.4, acc[:, :], ALU.mult, ALU.add)
        v.scalar_tensor_tensor(acc[:, :], cnt3[:, :], 0.2, acc[:, :], ALU.mult, ALU.add)
        v.scalar_tensor_tensor(acc[:, :], R4[:, :], 0.1, acc[:, :], ALU.mult, ALU.add)
        base = smt("base")
        v.tensor_scalar(base[:, :], acc[:, :], 100.0 / S, 0.1 * 100.0 / S,
                        ALU.mult, ALU.add)
        scores = smt("scores")
        v.tensor_mul(scores[:, :], base[:, :], mlt[:, :])

        ev = smt("ev")
        v.tensor_scalar(ev[:, :], scores[:, :], 30.0, None, ALU.is_lt)
        dpi = smt("dpi")
        v.tensor_scalar(dpi[:, :], scores[:, :], 15.0, -15.0, ALU.max, ALU.add)

        sd = smt("sd")
        v.tensor_sub(sd[:, :], dvec[:, :], td[:, :])
        sda = smt("sda")
        nc.scalar.activation(sda[:, :], sd[:, :], ACTF.Abs)
        sp = smt("sp")
        v.tensor_sub(sp[:, :], pvec[:, :], tp[:, :])
        spa = smt("spa")
        nc.scalar.activation(spa[:, :], sp[:, :], ACTF.Abs)
        sim = smt("sim")
        v.tensor_add(sim[:, :], sda[:, :], spa[:, :])

        ed = smt("ed")
        v.tensor_scalar(ed[:, :], dvec[:, :], 20.0, -20.0, ALU.max, ALU.add)
        ep = smt("ep")
        v.tensor_scalar(ep[:, :], pvec[:, :], 0.3, -0.3, ALU.max, ALU.add)
        eff = smt("eff")
        v.scalar_tensor_tensor(eff[:, :], ed[:, :], 1.0 / 20.0, ep[:, :],
                               ALU.mult, ALU.add)

        cd = smt("cd")
        v.tensor_sub(cd[:, :], cvec[:, :], ev[:, :])
        cq = smt("cq")
        v.tensor_mul(cq[:, :], cd[:, :], cd[:, :])

        ctot = smt("ctot")
        v.tensor_scalar(ctot[:, :], dpi[:, :], 2.0 / 30.0, None, ALU.mult)
        v.scalar_tensor_tensor(ctot[:, :], sim[:, :], 0.5, ctot[:, :], ALU.mult, ALU.add)
        v.scalar_tensor_tensor(ctot[:, :], eff[:, :], 0.3, ctot[:, :], ALU.mult, ALU.add)
        v.scalar_tensor_tensor(ctot[:, :], cq[:, :], 0.2, ctot[:, :], ALU.mult, ALU.add)

        red = sm.tile([P, 1], F32, tag="red", name="red")
        v.tensor_reduce(red[:, :], ctot[:, :], axis=mybir.AxisListType.X, op=ALU.add)
        nc.sync.dma_start(out_h[:, :], red[:, :])

    nc.finalize()
    return nc


def _get_nc() -> bass.Bass:
    global _NC_CACHE
    if _NC_CACHE is None:
        _NC_CACHE = _build_nc()
    return _NC_CACHE


def kernel(raw_sizes, raw_delays, raw_directions, delay_ms, padding_norm,
           confidence, profile_ids, trace=False, tmpdir=None):
    global LAST_RESULTS
    raw_sizes = np.asarray(raw_sizes, dtype=np.int32)
    raw_delays = np.asarray(raw_delays, dtype=np.float32)
    raw_directions = np.asarray(raw_directions, dtype=np.int32)
    delay_ms = np.asarray(delay_ms, dtype=np.float32)
    padding_norm = np.asarray(padding_norm, dtype=np.float32)
    confidence = np.asarray(confidence, dtype=np.float32)
    profile_ids = np.asarray(profile_ids).astype(np.int32)

    nc = _get_nc()
    in_maps = []
    for i in range(N_CORES):
        r = slice(i * BC, (i + 1) * BC)
        in_maps.append({
            "raw_sizes": raw_sizes[r],
            "raw_delays": raw_delays[r],
            "raw_directions": raw_directions[r],
            "delay_ms": delay_ms[r],
            "padding_norm": padding_norm[r],
            "confidence": confidence[r],
            "profile_ids": profile_ids[r],
        })

    LAST_RESULTS = run_bass_kernel_spmd(nc, in_maps, list(range(N_CORES)),
                                        trace=trace, tmpdir=tmpdir)
    partials = [LAST_RESULTS.results[i]["partial"] for i in range(N_CORES)]
    total = float(np.sum(np.stack(partials), dtype=np.float64))
    return np.float32(total / B)


# revision 16
# speedup vs baseline: 1.1038x; 1.1038x over previous
"""AdversarialMorphingLoss — Trainium2 Bass kernel (8-core data parallel).

Full inputs arrive on the host; we shard the batch dim (B=4096) into 8
contiguous blocks of 512 rows, run one SPMD Bass program on all 8
NeuronCores, and each core returns the partial (un-normalized) sum of the
per-sample loss contribution over its 512 rows.  The host sums the 8
partials and divides by B.

Per-sample math (matching reference.py):
  scores_b = 100/S * sum_s inc_s * CONFIG_MULT[pid % 4]
  inc_s    = 0.6*(sz_s > 1400) + 0.4*(dly_s < 0.05)
           + 0.2*(|sz_s - sz_{s-1}| < 0.5) + 0.1*(dir_s != dir_{s-1})
  with sz[:, -1] -> min(sz[:, -1] + pad*1500, 1500), dly[:, -1] += delay_ms,
  and the s=0 "prev" being -1.0 (so the dir term contributes 0.1 at s=0 and
  the size-equality term contributes 0).

  c_b = (2/30)*relu(scores-15) + 0.5*(|dly_ms - TD[pid]| + |pad - TP[pid]|)
      + 0.3*(relu(dly_ms-20)/20 + relu(pad-0.3)) + 0.2*(conf - (scores<30))^2
  loss = mean_b c_b

On-device strategy (memory-bound: streams 96 MB of traces):
  * count (sz > 1400) over all S int32 cols with one ScalarE
    activation(Sign, bias=-1400.5, accum_out=...) per [128, 2048] tile
    (integers never hit the .5 threshold -> exact), then patch the last
    (float-modified) column with exact [128,4] is_gt ops.
  * count (dly < 0.05) the same way via Sign(0.05 - dly).
  * count consecutive-size equality / direction flips with one fused
    VectorE tensor_tensor_reduce(is_equal / not_equal, accum_out=...) per
    tile, again patching the last column separately.
  * everything per-sample afterwards runs on tiny [128, 4] tiles.
"""

import numpy as np
from contextlib import ExitStack

import concourse.bass as bass
import concourse.bacc as bacc
import concourse.mybir as mybir
from concourse import tile
from concourse.bass_utils import run_bass_kernel_spmd

B, S = 4096, 2048
N_CORES = 8
BC = B // N_CORES          # 512 rows per core
P = 128                    # SBUF partitions
NT = BC // P               # 4 tiles of 128 rows per core

F32 = mybir.dt.float32
I32 = mybir.dt.int32
ALU = mybir.AluOpType
ACTF = mybir.ActivationFunctionType

_NC_CACHE = None
LAST_RESULTS = None        # BassKernelResults of the last kernel() call


def _build_nc() -> bass.Bass:
    nc = bacc.Bacc()

    sz_h = nc.declare_dram_parameter("raw_sizes", [BC, S], I32, isOutput=False)
    dl_h = nc.declare_dram_parameter("raw_delays", [BC, S], F32, isOutput=False)
    dr_h = nc.declare_dram_parameter("raw_directions", [BC, S], I32, isOutput=False)
    dms_h = nc.declare_dram_parameter("delay_ms", [BC], F32, isOutput=False)
    pad_h = nc.declare_dram_parameter("padding_norm", [BC], F32, isOutput=False)
    cnf_h = nc.declare_dram_parameter("confidence", [BC], F32, isOutput=False)
    pid_h = nc.declare_dram_parameter("profile_ids", [BC], I32, isOutput=False)
    out_h = nc.declare_dram_parameter("partial", [P, 1], F32, isOutput=True)

    with tile.TileContext(nc) as tc, ExitStack() as ctx:
        inp = ctx.enter_context(tc.tile_pool(name="inp", bufs=4))
        scr = ctx.enter_context(tc.tile_pool(name="scr", bufs=2))
        sm = ctx.enter_context(tc.tile_pool(name="sm", bufs=1))

        def smt(tag, dtype=F32):
            return sm.tile([P, NT], dtype, tag=tag, name=tag)

        _consts = {}

        def constv(val):
            """[128,1] f32 SBUF tile holding `val` (for activation bias APs)."""
            if val not in _consts:
                cname = f"cst{len(_consts)}"
                ct = sm.tile([P, 1], F32, tag=cname, name=cname)
                nc.vector.memset(ct[:, :], val)
                _consts[val] = ct[:, :]
            return _consts[val]

        # Per-row (per-sample) vectors, laid out [partition, tile]: (p, t) = v[t*128 + p]
        dvec = smt("dvec")
        pvec = smt("pvec")
        cvec = smt("cvec")
        pidt = smt("pidt", I32)

        # Big-op accumulators (per tile column)
        R1 = smt("R1")     # sum sign(sz - 1400.5)           over all S cols
        R2 = smt("R2")     # sum sign(0.05 - dly)            over all S cols
        R3 = smt("R3")     # count sz[s] == sz[s-1],  s=1..S-1 (unmodified last col)
        R4 = smt("R4")     # count dir[s] != dir[s-1], s=1..S-1
        szlast = smt("szlast")   # f32 copy of int sz[:, S-1]
        szprev = smt("szprev")   # f32 copy of int sz[:, S-2]
        dllast = smt("dllast")   # f32 copy of dly[:, S-1]

        for t in range(NT):
            rows = slice(t * P, (t + 1) * P)
            szt = inp.tile([P, S], I32, tag="szt")
            dlt = inp.tile([P, S], F32, tag="dlt")
            drt = inp.tile([P, S], I32, tag="drt")
            nc.sync.dma_start(szt[:, :], sz_h[rows, :])
            nc.sync.dma_start(dlt[:, :], dl_h[rows, :])
            nc.sync.dma_start(drt[:, :], dr_h[rows, :])

            col = slice(t, t + 1)
            o1 = scr.tile([P, S], F32, tag="o1")
            nc.scalar.activation(o1[:, :], szt[:, :], ACTF.Sign,
                                 bias=constv(-1400.5), scale=1.0, accum_out=R1[:, col])
            o2 = scr.tile([P, S], F32, tag="o2")
            nc.scalar.activation(o2[:, :], dlt[:, :], ACTF.Sign,
                                 bias=constv(0.05), scale=-1.0, accum_out=R2[:, col])
            # fused compare + row-sum on DVE: out = (in0 bypass 0) cmp in1,
            # accum_out = sum(out).  (tensor_tensor_reduce crashes the HW
            # runtime in this toolchain; scalar_tensor_tensor w/ accum works.)
            o3 = scr.tile([P, S - 1], F32, tag="o3")
            nc.vector.scalar_tensor_tensor(
                o3[:, :], szt[:, 1:S], 0.0, szt[:, 0:S - 1],
                ALU.bypass, ALU.is_equal, accum_out=R3[:, col])
            o4 = scr.tile([P, S - 1], F32, tag="o4")
            nc.vector.scalar_tensor_tensor(
                o4[:, :], drt[:, 1:S], 0.0, drt[:, 0:S - 1],
                ALU.bypass, ALU.not_equal, accum_out=R4[:, col])

            nc.vector.tensor_copy(szlast[:, col], szt[:, S - 1:S])
            nc.vector.tensor_copy(szprev[:, col], szt[:, S - 2:S - 1])
            nc.vector.tensor_copy(dllast[:, col], dlt[:, S - 1:S])

        # per-row vectors loaded after the big streams are queued (tiny DMAs)
        nc.sync.dma_start(dvec[:, :], dms_h[:].rearrange("(t p) -> p t", p=P))
        nc.sync.dma_start(pvec[:, :], pad_h[:].rearrange("(t p) -> p t", p=P))
        nc.sync.dma_start(cvec[:, :], cnf_h[:].rearrange("(t p) -> p t", p=P))
        nc.sync.dma_start(pidt[:, :], pid_h[:].rearrange("(t p) -> p t", p=P))

        # ---- per-sample combine, all on [128, 4] tiles (VectorE only,
        # to keep per-instruction sync-wait counts low on ScalarE) ----
        v = nc.vector

        # profile-id one-hots (pid in 0..4)
        pidf = smt("pidf")
        v.tensor_copy(pidf[:, :], pidt[:, :])
        e1, e2, e3, e4 = smt("e1"), smt("e2"), smt("e3"), smt("e4")
        v.tensor_scalar(e1[:, :], pidf[:, :], 1.0, None, ALU.is_equal)
        v.tensor_scalar(e2[:, :], pidf[:, :], 2.0, None, ALU.is_equal)
        v.tensor_scalar(e3[:, :], pidf[:, :], 3.0, None, ALU.is_equal)
        v.tensor_scalar(e4[:, :], pidf[:, :], 4.0, None, ALU.is_equal)

        # CONFIG_MULT[pid % 4] = 1.0 + 0.3*e1 + 0.6*e2 + 1.0*e3  (pid=4 -> 1.0)
        mlt = smt("mlt")
        v.tensor_scalar(mlt[:, :], e1[:, :], 0.3, 1.0, ALU.mult, ALU.add)
        v.scalar_tensor_tensor(mlt[:, :], e2[:, :], 0.6, mlt[:, :], ALU.mult, ALU.add)
        v.tensor_add(mlt[:, :], mlt[:, :], e3[:, :])

        # TARGET_DELAY[pid] = 2 - 1*e1 - 1.5*e2 + 3*e3 + 1*e4
        td = smt("td")
        v.tensor_scalar(td[:, :], e1[:, :], -1.0, 2.0, ALU.mult, ALU.add)
        v.scalar_tensor_tensor(td[:, :], e2[:, :], -1.5, td[:, :], ALU.mult, ALU.add)
        v.scalar_tensor_tensor(td[:, :], e3[:, :], 3.0, td[:, :], ALU.mult, ALU.add)
        v.tensor_add(td[:, :], td[:, :], e4[:, :])

        # TARGET_PAD[pid] = 0.08 + 0.04*e1 - 0.03*e2 + 0.07*e3 + 0.02*e4
        tp = smt("tp")
        v.tensor_scalar(tp[:, :], e1[:, :], 0.04, 0.08, ALU.mult, ALU.add)
        v.scalar_tensor_tensor(tp[:, :], e2[:, :], -0.03, tp[:, :], ALU.mult, ALU.add)
        v.scalar_tensor_tensor(tp[:, :], e3[:, :], 0.07, tp[:, :], ALU.mult, ALU.add)
        v.scalar_tensor_tensor(tp[:, :], e4[:, :], 0.02, tp[:, :], ALU.mult, ALU.add)

        # last-column morphing fixups
        padx = smt("padx")
        v.tensor_scalar(padx[:, :], pvec[:, :], 1500.0, None, ALU.mult)
        szmod = smt("szmod")
        v.tensor_add(szmod[:, :], szlast[:, :], padx[:, :])
        v.tensor_scalar(szmod[:, :], szmod[:, :], 1500.0, None, ALU.min)
        dlmod = smt("dlmod")
        v.tensor_add(dlmod[:, :], dllast[:, :], dvec[:, :])

        g1m, g1r = smt("g1m"), smt("g1r")
        v.tensor_scalar(g1m[:, :], szmod[:, :], 1400.0, None, ALU.is_gt)
        v.tensor_scalar(g1r[:, :], szlast[:, :], 1400.0, None, ALU.is_gt)
        l2m, l2r = smt("l2m"), smt("l2r")
        v.tensor_scalar(l2m[:, :], dlmod[:, :], 0.05, None, ALU.is_lt)
        v.tensor_scalar(l2r[:, :], dllast[:, :], 0.05, None, ALU.is_lt)
        e3r = smt("e3r")
        v.tensor_tensor(e3r[:, :], szlast[:, :], szprev[:, :], ALU.is_equal)
        d3 = smt("d3")
        v.tensor_sub(d3[:, :], szmod[:, :], szprev[:, :])
        a3 = smt("a3")
        nc.scalar.activation(a3[:, :], d3[:, :], ACTF.Abs)
        e3m = smt("e3m")
        v.tensor_scalar(e3m[:, :], a3[:, :], 0.5, None, ALU.is_lt)

        # exact per-row counts
        cnt1 = smt("cnt1")
        v.tensor_scalar(cnt1[:, :], R1[:, :], 0.5, float(S) / 2, ALU.mult, ALU.add)
        v.tensor_sub(cnt1[:, :], cnt1[:, :], g1r[:, :])
        v.tensor_add(cnt1[:, :], cnt1[:, :], g1m[:, :])
        cnt2 = smt("cnt2")
        v.tensor_scalar(cnt2[:, :], R2[:, :], 0.5, float(S) / 2, ALU.mult, ALU.add)
        v.tensor_sub(cnt2[:, :], cnt2[:, :], l2r[:, :])
        v.tensor_add(cnt2[:, :], cnt2[:, :], l2m[:, :])
        cnt3 = smt("cnt3")
        v.tensor_sub(cnt3[:, :], R3[:, :], e3r[:, :])
        v.tensor_add(cnt3[:, :], cnt3[:, :], e3m[:, :])

        # scores = (0.6*c1 + 0.4*c2 + 0.2*c3 + 0.1*c4 + 0.1) * (100/S) * mult
        acc = smt("acc")
        v.tensor_scalar(acc[:, :], cnt1[:, :], 0.6, None, ALU.mult)
        v.scalar_tensor_tensor(acc[:, :], cnt2[:, :], 0.4, acc[:, :], ALU.mult, ALU.add)
        v.scalar_tensor_tensor(acc[:, :], cnt3[:, :], 0.2, acc[:, :], ALU.mult, ALU.add)
        v.scalar_tensor_tensor(acc[:, :], R4[:, :], 0.1, acc[:, :], ALU.mult, ALU.add)
        base = smt("base")
        v.tensor_scalar(base[:, :], acc[:, :], 100.0 / S, 0.1 * 100.0 / S,
                        ALU.mult, ALU.add)
        scores = smt("scores")
        v.tensor_mul(scores[:, :], base[:, :], mlt[:, :])

        ev = smt("ev")
        v.tensor_scalar(ev[:, :], scores[:, :], 30.0, None, ALU.is_lt)
        dpi = smt("dpi")
        v.tensor_scalar(dpi[:, :], scores[:, :], 15.0, -15.0, ALU.max, ALU.add)

        sd = smt("sd")
        v.tensor_sub(sd[:, :], dvec[:, :], td[:, :])
        sda = smt("sda")
        nc.scalar.activation(sda[:, :], sd[:, :], ACTF.Abs)
        sp = smt("sp")
        v.tensor_sub(sp[:, :], pvec[:, :], tp[:, :])
        spa = smt("spa")
        nc.scalar.activation(spa[:, :], sp[:, :], ACTF.Abs)
        sim = smt("sim")
        v.tensor_add(sim[:, :], sda[:, :], spa[:, :])

        ed = smt("ed")
        v.tensor_scalar(ed[:, :], dvec[:, :], 20.0, -20.0, ALU.max, ALU.add)
        ep = smt("ep")
        v.tensor_scalar(ep[:, :], pvec[:, :], 0.3, -0.3, ALU.max, ALU.add)
        eff = smt("eff")
        v.scalar_tensor_tensor(eff[:, :], ed[:, :], 1.0 / 20.0, ep[:, :],
                               ALU.mult, ALU.add)

        cd = smt("cd")
        v.tensor_sub(cd[:, :], cvec[:, :], ev[:, :])
        cq = smt("cq")
        v.tensor_mul(cq[:, :], cd[:, :], cd[:, :])

        ctot = smt("ctot")
        v.tensor_scalar(ctot[:, :], dpi[:, :], 2.0 / 30.0, None, ALU.mult)
        v.scalar_tensor_tensor(ctot[:, :], sim[:, :], 0.5, ctot[:, :], ALU.mult, ALU.add)
        v.scalar_tensor_tensor(ctot[:, :], eff[:, :], 0.3, ctot[:, :], ALU.mult, ALU.add)
        v.scalar_tensor_tensor(ctot[:, :], cq[:, :], 0.2, ctot[:, :], ALU.mult, ALU.add)

        red = sm.tile([P, 1], F32, tag="red", name="red")
        v.tensor_reduce(red[:, :], ctot[:, :], axis=mybir.AxisListType.X, op=ALU.add)
        nc.sync.dma_start(out_h[:, :], red[:, :])

    nc.finalize()
    return nc


def _get_nc() -> bass.Bass:
    global _NC_CACHE
    if _NC_CACHE is None:
        _NC_CACHE = _build_nc()
    return _NC_CACHE


def kernel(raw_sizes, raw_delays, raw_directions, delay_ms, padding_norm,
           confidence, profile_ids, trace=False, tmpdir=None):
    global LAST_RESULTS
    raw_sizes = np.asarray(raw_sizes, dtype=np.int32)
    raw_delays = np.asarray(raw_delays, dtype=np.float32)
    raw_directions = np.asarray(raw_directions, dtype=np.int32)
    delay_ms = np.asarray(delay_ms, dtype=np.float32)
    padding_norm = np.asarray(padding_norm, dtype=np.float32)
    confidence = np.asarray(confidence, dtype=np.float32)
    profile_ids = np.asarray(profile_ids).astype(np.int32)

    nc = _get_nc()
    in_maps = []
    for i in range(N_CORES):
        r = slice(i * BC, (i + 1) * BC)
        in_maps.append({
            "raw_sizes": raw_sizes[r],
            "raw_delays": raw_delays[r],
            "raw_directions": raw_directions[r],
            "delay_ms": delay_ms[r],
            "padding_norm": padding_norm[r],
            "confidence": confidence[r],
            "profile_ids": profile_ids[r],
        })

    LAST_RESULTS = run_bass_kernel_spmd(nc, in_maps, list(range(N_CORES)),
                                        trace=trace, tmpdir=tmpdir)
    partials = [LAST_RESULTS.results[i]["partial"] for i in range(N_CORES)]
    total = float(np.sum(np.stack(partials), dtype=np.float64))
    return np.float32(total / B)


# revision 17
# speedup vs baseline: 1.1047x; 1.0008x over previous
"""AdversarialMorphingLoss — Trainium2 Bass kernel (8-core data parallel).

Full inputs arrive on the host; we shard the batch dim (B=4096) into 8
contiguous blocks of 512 rows, run one SPMD Bass program on all 8
NeuronCores, and each core returns the partial (un-normalized) sum of the
per-sample loss contribution over its 512 rows.  The host sums the 8
partials and divides by B.

Per-sample math (matching reference.py):
  scores_b = 100/S * sum_s inc_s * CONFIG_MULT[pid % 4]
  inc_s    = 0.6*(sz_s > 1400) + 0.4*(dly_s < 0.05)
           + 0.2*(|sz_s - sz_{s-1}| < 0.5) + 0.1*(dir_s != dir_{s-1})
  with sz[:, -1] -> min(sz[:, -1] + pad*1500, 1500), dly[:, -1] += delay_ms,
  and the s=0 "prev" being -1.0 (so the dir term contributes 0.1 at s=0 and
  the size-equality term contributes 0).

  c_b = (2/30)*relu(scores-15) + 0.5*(|dly_ms - TD[pid]| + |pad - TP[pid]|)
      + 0.3*(relu(dly_ms-20)/20 + relu(pad-0.3)) + 0.2*(conf - (scores<30))^2
  loss = mean_b c_b

On-device strategy (memory-bound: streams 96 MB of traces):
  * count (sz > 1400) over all S int32 cols with one ScalarE
    activation(Sign, bias=-1400.5, accum_out=...) per [128, 2048] tile
    (integers never hit the .5 threshold -> exact), then patch the last
    (float-modified) column with exact [128,4] is_gt ops.
  * count (dly < 0.05) the same way via Sign(0.05 - dly).
  * count consecutive-size equality / direction flips with one fused
    VectorE tensor_tensor_reduce(is_equal / not_equal, accum_out=...) per
    tile, again patching the last column separately.
  * everything per-sample afterwards runs on tiny [128, 4] tiles.
"""

import numpy as np
from contextlib import ExitStack

import concourse.bass as bass
import concourse.bacc as bacc
import concourse.mybir as mybir
from concourse import tile
from concourse.bass_utils import run_bass_kernel_spmd

B, S = 4096, 2048
N_CORES = 8
BC = B // N_CORES          # 512 rows per core
P = 128                    # SBUF partitions
NT = BC // P               # 4 tiles of 128 rows per core

F32 = mybir.dt.float32
I32 = mybir.dt.int32
ALU = mybir.AluOpType
ACTF = mybir.ActivationFunctionType

_NC_CACHE = None
LAST_RESULTS = None        # BassKernelResults of the last kernel() call


def _build_nc() -> bass.Bass:
    nc = bacc.Bacc()

    sz_h = nc.declare_dram_parameter("raw_sizes", [BC, S], I32, isOutput=False)
    dl_h = nc.declare_dram_parameter("raw_delays", [BC, S], F32, isOutput=False)
    dr_h = nc.declare_dram_parameter("raw_directions", [BC, S], I32, isOutput=False)
    dms_h = nc.declare_dram_parameter("delay_ms", [BC], F32, isOutput=False)
    pad_h = nc.declare_dram_parameter("padding_norm", [BC], F32, isOutput=False)
    cnf_h = nc.declare_dram_parameter("confidence", [BC], F32, isOutput=False)
    pid_h = nc.declare_dram_parameter("profile_ids", [BC], I32, isOutput=False)
    out_h = nc.declare_dram_parameter("partial", [P, 1], F32, isOutput=True)

    with tile.TileContext(nc) as tc, ExitStack() as ctx:
        inp = ctx.enter_context(tc.tile_pool(name="inp", bufs=4))
        scr = ctx.enter_context(tc.tile_pool(name="scr", bufs=2))
        sm = ctx.enter_context(tc.tile_pool(name="sm", bufs=1))

        def smt(tag, dtype=F32):
            return sm.tile([P, NT], dtype, tag=tag, name=tag)

        _consts = {}

        def constv(val):
            """[128,1] f32 SBUF tile holding `val` (for activation bias APs)."""
            if val not in _consts:
                cname = f"cst{len(_consts)}"
                ct = sm.tile([P, 1], F32, tag=cname, name=cname)
                nc.vector.memset(ct[:, :], val)
                _consts[val] = ct[:, :]
            return _consts[val]

        # Row mapping: core row r -> (partition p, tile t) with r = p*NT + t.
        # This makes the per-row [128, NT] vector loads a dense 2D DMA
        # (partition stride 16B) instead of a 512-descriptor gather, while
        # the big tile loads just become row-strided (stride NT rows), which
        # costs the same descriptors as contiguous rows.
        dvec = smt("dvec")
        pvec = smt("pvec")
        cvec = smt("cvec")
        pidt = smt("pidt", I32)

        # Big-op accumulators (per tile column)
        R1 = smt("R1")     # sum sign(sz - 1400.5)           over all S cols
        R2 = smt("R2")     # sum sign(0.05 - dly)            over all S cols
        R3 = smt("R3")     # count sz[s] == sz[s-1],  s=1..S-1 (unmodified last col)
        R4 = smt("R4")     # count dir[s] != dir[s-1], s=1..S-1
        szlast = smt("szlast")   # f32 copy of int sz[:, S-1]
        szprev = smt("szprev")   # f32 copy of int sz[:, S-2]
        dllast = smt("dllast")   # f32 copy of dly[:, S-1]

        sz_t = sz_h[:, :].rearrange("(p t) s -> t p s", t=NT)
        dl_t = dl_h[:, :].rearrange("(p t) s -> t p s", t=NT)
        dr_t = dr_h[:, :].rearrange("(p t) s -> t p s", t=NT)
        for t in range(NT):
            szt = inp.tile([P, S], I32, tag="szt")
            dlt = inp.tile([P, S], F32, tag="dlt")
            drt = inp.tile([P, S], I32, tag="drt")
            nc.sync.dma_start(szt[:, :], sz_t[t])
            nc.sync.dma_start(dlt[:, :], dl_t[t])
            nc.sync.dma_start(drt[:, :], dr_t[t])

            col = slice(t, t + 1)
            o1 = scr.tile([P, S], F32, tag="o1")
            nc.scalar.activation(o1[:, :], szt[:, :], ACTF.Sign,
                                 bias=constv(-1400.5), scale=1.0, accum_out=R1[:, col])
            o2 = scr.tile([P, S], F32, tag="o2")
            nc.scalar.activation(o2[:, :], dlt[:, :], ACTF.Sign,
                                 bias=constv(0.05), scale=-1.0, accum_out=R2[:, col])
            # fused compare + row-sum on DVE: out = (in0 bypass 0) cmp in1,
            # accum_out = sum(out).  (tensor_tensor_reduce crashes the HW
            # runtime in this toolchain; scalar_tensor_tensor w/ accum works.)
            o3 = scr.tile([P, S - 1], F32, tag="o3")
            nc.vector.scalar_tensor_tensor(
                o3[:, :], szt[:, 1:S], 0.0, szt[:, 0:S - 1],
                ALU.bypass, ALU.is_equal, accum_out=R3[:, col])
            o4 = scr.tile([P, S - 1], F32, tag="o4")
            nc.vector.scalar_tensor_tensor(
                o4[:, :], drt[:, 1:S], 0.0, drt[:, 0:S - 1],
                ALU.bypass, ALU.not_equal, accum_out=R4[:, col])

            nc.vector.tensor_copy(szlast[:, col], szt[:, S - 1:S])
            nc.vector.tensor_copy(szprev[:, col], szt[:, S - 2:S - 1])
            nc.vector.tensor_copy(dllast[:, col], dlt[:, S - 1:S])

        # per-row vectors loaded after the big streams are queued (tiny DMAs)
        nc.sync.dma_start(dvec[:, :], dms_h[:].rearrange("(p t) -> p t", t=NT))
        nc.sync.dma_start(pvec[:, :], pad_h[:].rearrange("(p t) -> p t", t=NT))
        nc.sync.dma_start(cvec[:, :], cnf_h[:].rearrange("(p t) -> p t", t=NT))
        nc.sync.dma_start(pidt[:, :], pid_h[:].rearrange("(p t) -> p t", t=NT))

        # ---- per-sample combine, all on [128, 4] tiles (VectorE only,
        # to keep per-instruction sync-wait counts low on ScalarE) ----
        v = nc.vector

        # profile-id one-hots (pid in 0..4)
        pidf = smt("pidf")
        v.tensor_copy(pidf[:, :], pidt[:, :])
        e1, e2, e3, e4 = smt("e1"), smt("e2"), smt("e3"), smt("e4")
        v.tensor_scalar(e1[:, :], pidf[:, :], 1.0, None, ALU.is_equal)
        v.tensor_scalar(e2[:, :], pidf[:, :], 2.0, None, ALU.is_equal)
        v.tensor_scalar(e3[:, :], pidf[:, :], 3.0, None, ALU.is_equal)
        v.tensor_scalar(e4[:, :], pidf[:, :], 4.0, None, ALU.is_equal)

        # CONFIG_MULT[pid % 4] = 1.0 + 0.3*e1 + 0.6*e2 + 1.0*e3  (pid=4 -> 1.0)
        mlt = smt("mlt")
        v.tensor_scalar(mlt[:, :], e1[:, :], 0.3, 1.0, ALU.mult, ALU.add)
        v.scalar_tensor_tensor(mlt[:, :], e2[:, :], 0.6, mlt[:, :], ALU.mult, ALU.add)
        v.tensor_add(mlt[:, :], mlt[:, :], e3[:, :])

        # TARGET_DELAY[pid] = 2 - 1*e1 - 1.5*e2 + 3*e3 + 1*e4
        td = smt("td")
        v.tensor_scalar(td[:, :], e1[:, :], -1.0, 2.0, ALU.mult, ALU.add)
        v.scalar_tensor_tensor(td[:, :], e2[:, :], -1.5, td[:, :], ALU.mult, ALU.add)
        v.scalar_tensor_tensor(td[:, :], e3[:, :], 3.0, td[:, :], ALU.mult, ALU.add)
        v.tensor_add(td[:, :], td[:, :], e4[:, :])

        # TARGET_PAD[pid] = 0.08 + 0.04*e1 - 0.03*e2 + 0.07*e3 + 0.02*e4
        tp = smt("tp")
        v.tensor_scalar(tp[:, :], e1[:, :], 0.04, 0.08, ALU.mult, ALU.add)
        v.scalar_tensor_tensor(tp[:, :], e2[:, :], -0.03, tp[:, :], ALU.mult, ALU.add)
        v.scalar_tensor_tensor(tp[:, :], e3[:, :], 0.07, tp[:, :], ALU.mult, ALU.add)
        v.scalar_tensor_tensor(tp[:, :], e4[:, :], 0.02, tp[:, :], ALU.mult, ALU.add)

        # last-column morphing fixups
        padx = smt("padx")
        v.tensor_scalar(padx[:, :], pvec[:, :], 1500.0, None, ALU.mult)
        szmod = smt("szmod")
        v.tensor_add(szmod[:, :], szlast[:, :], padx[:, :])
        v.tensor_scalar(szmod[:, :], szmod[:, :], 1500.0, None, ALU.min)
        dlmod = smt("dlmod")
        v.tensor_add(dlmod[:, :], dllast[:, :], dvec[:, :])

        g1m, g1r = smt("g1m"), smt("g1r")
        v.tensor_scalar(g1m[:, :], szmod[:, :], 1400.0, None, ALU.is_gt)
        v.tensor_scalar(g1r[:, :], szlast[:, :], 1400.0, None, ALU.is_gt)
        l2m, l2r = smt("l2m"), smt("l2r")
        v.tensor_scalar(l2m[:, :], dlmod[:, :], 0.05, None, ALU.is_lt)
        v.tensor_scalar(l2r[:, :], dllast[:, :], 0.05, None, ALU.is_lt)
        e3r = smt("e3r")
        v.tensor_tensor(e3r[:, :], szlast[:, :], szprev[:, :], ALU.is_equal)
        d3 = smt("d3")
        v.tensor_sub(d3[:, :], szmod[:, :], szprev[:, :])
        a3 = smt("a3")
        nc.scalar.activation(a3[:, :], d3[:, :], ACTF.Abs)
        e3m = smt("e3m")
        v.tensor_scalar(e3m[:, :], a3[:, :], 0.5, None, ALU.is_lt)

        # exact per-row counts
        cnt1 = smt("cnt1")
        v.tensor_scalar(cnt1[:, :], R1[:, :], 0.5, float(S) / 2, ALU.mult, ALU.add)
        v.tensor_sub(cnt1[:, :], cnt1[:, :], g1r[:, :])
        v.tensor_add(cnt1[:, :], cnt1[:, :], g1m[:, :])
        cnt2 = smt("cnt2")
        v.tensor_scalar(cnt2[:, :], R2[:, :], 0.5, float(S) / 2, ALU.mult, ALU.add)
        v.tensor_sub(cnt2[:, :], cnt2[:, :], l2r[:, :])
        v.tensor_add(cnt2[:, :], cnt2[:, :], l2m[:, :])
        cnt3 = smt("cnt3")
        v.tensor_sub(cnt3[:, :], R3[:, :], e3r[:, :])
        v.tensor_add(cnt3[:, :], cnt3[:, :], e3m[:, :])

        # scores = (0.6*c1 + 0.4*c2 + 0.2*c3 + 0.1*c4 + 0.1) * (100/S) * mult
        acc = smt("acc")
        v.tensor_scalar(acc[:, :], cnt1[:, :], 0.6, None, ALU.mult)
        v.scalar_tensor_tensor(acc[:, :], cnt2[:, :], 0.4, acc[:, :], ALU.mult, ALU.add)
        v.scalar_tensor_tensor(acc[:, :], cnt3[:, :], 0.2, acc[:, :], ALU.mult, ALU.add)
        v.scalar_tensor_tensor(acc[:, :], R4[:, :], 0.1, acc[:, :], ALU.mult, ALU.add)
        base = smt("base")
        v.tensor_scalar(base[:, :], acc[:, :], 100.0 / S, 0.1 * 100.0 / S,
                        ALU.mult, ALU.add)
        scores = smt("scores")
        v.tensor_mul(scores[:, :], base[:, :], mlt[:, :])

        ev = smt("ev")
        v.tensor_scalar(ev[:, :], scores[:, :], 30.0, None, ALU.is_lt)
        dpi = smt("dpi")
        v.tensor_scalar(dpi[:, :], scores[:, :], 15.0, -15.0, ALU.max, ALU.add)

        sd = smt("sd")
        v.tensor_sub(sd[:, :], dvec[:, :], td[:, :])
        sda = smt("sda")
        nc.scalar.activation(sda[:, :], sd[:, :], ACTF.Abs)
        sp = smt("sp")
        v.tensor_sub(sp[:, :], pvec[:, :], tp[:, :])
        spa = smt("spa")
        nc.scalar.activation(spa[:, :], sp[:, :], ACTF.Abs)
        sim = smt("sim")
        v.tensor_add(sim[:, :], sda[:, :], spa[:, :])

        ed = smt("ed")
        v.tensor_scalar(ed[:, :], dvec[:, :], 20.0, -20.0, ALU.max, ALU.add)
        ep = smt("ep")
        v.tensor_scalar(ep[:, :], pvec[:, :], 0.3, -0.3, ALU.max, ALU.add)
        eff = smt("eff")
        v.scalar_tensor_tensor(eff[:, :], ed[:, :], 1.0 / 20.0, ep[:, :],
                               ALU.mult, ALU.add)

        cd = smt("cd")
        v.tensor_sub(cd[:, :], cvec[:, :], ev[:, :])
        cq = smt("cq")
        v.tensor_mul(cq[:, :], cd[:, :], cd[:, :])

        ctot = smt("ctot")
        v.tensor_scalar(ctot[:, :], dpi[:, :], 2.0 / 30.0, None, ALU.mult)
        v.scalar_tensor_tensor(ctot[:, :], sim[:, :], 0.5, ctot[:, :], ALU.mult, ALU.add)
        v.scalar_tensor_tensor(ctot[:, :], eff[:, :], 0.3, ctot[:, :], ALU.mult, ALU.add)
        v.scalar_tensor_tensor(ctot[:, :], cq[:, :], 0.2, ctot[:, :], ALU.mult, ALU.add)

        red = sm.tile([P, 1], F32, tag="red", name="red")
        v.tensor_reduce(red[:, :], ctot[:, :], axis=mybir.AxisListType.X, op=ALU.add)
        nc.sync.dma_start(out_h[:, :], red[:, :])

    nc.finalize()
    return nc


def _get_nc() -> bass.Bass:
    global _NC_CACHE
    if _NC_CACHE is None:
        _NC_CACHE = _build_nc()
    return _NC_CACHE


def kernel(raw_sizes, raw_delays, raw_directions, delay_ms, padding_norm,
           confidence, profile_ids, trace=False, tmpdir=None):
    global LAST_RESULTS
    raw_sizes = np.asarray(raw_sizes, dtype=np.int32)
    raw_delays = np.asarray(raw_delays, dtype=np.float32)
    raw_directions = np.asarray(raw_directions, dtype=np.int32)
    delay_ms = np.asarray(delay_ms, dtype=np.float32)
    padding_norm = np.asarray(padding_norm, dtype=np.float32)
    confidence = np.asarray(confidence, dtype=np.float32)
    profile_ids = np.asarray(profile_ids).astype(np.int32)

    nc = _get_nc()
    in_maps = []
    for i in range(N_CORES):
        r = slice(i * BC, (i + 1) * BC)
        in_maps.append({
            "raw_sizes": raw_sizes[r],
            "raw_delays": raw_delays[r],
            "raw_directions": raw_directions[r],
            "delay_ms": delay_ms[r],
            "padding_norm": padding_norm[r],
            "confidence": confidence[r],
            "profile_ids": profile_ids[r],
        })

    LAST_RESULTS = run_bass_kernel_spmd(nc, in_maps, list(range(N_CORES)),
                                        trace=trace, tmpdir=tmpdir)
    partials = [LAST_RESULTS.results[i]["partial"] for i in range(N_CORES)]
    total = float(np.sum(np.stack(partials), dtype=np.float64))
    return np.float32(total / B)


# revision 18
# speedup vs baseline: 1.2473x; 1.1290x over previous
"""AdversarialMorphingLoss — Trainium2 Bass kernel (8-core data parallel).

Full inputs arrive on the host; we shard the batch dim (B=4096) into 8
contiguous blocks of 512 rows, run one SPMD Bass program on all 8
NeuronCores, and each core returns the partial (un-normalized) sum of the
per-sample loss contribution over its 512 rows.  The host sums the 8
partials and divides by B.

Per-sample math (matching reference.py):
  scores_b = 100/S * sum_s inc_s * CONFIG_MULT[pid % 4]
  inc_s    = 0.6*(sz_s > 1400) + 0.4*(dly_s < 0.05)
           + 0.2*(|sz_s - sz_{s-1}| < 0.5) + 0.1*(dir_s != dir_{s-1})
  with sz[:, -1] -> min(sz[:, -1] + pad*1500, 1500), dly[:, -1] += delay_ms,
  and the s=0 "prev" being -1.0 (so the dir term contributes 0.1 at s=0 and
  the size-equality term contributes 0).

  c_b = (2/30)*relu(scores-15) + 0.5*(|dly_ms - TD[pid]| + |pad - TP[pid]|)
      + 0.3*(relu(dly_ms-20)/20 + relu(pad-0.3)) + 0.2*(conf - (scores<30))^2
  loss = mean_b c_b

On-device strategy (memory-bound: streams 96 MB of traces):
  * count (sz > 1400) over all S int32 cols with one ScalarE
    activation(Sign, bias=-1400.5, accum_out=...) per [128, 2048] tile
    (integers never hit the .5 threshold -> exact), then patch the last
    (float-modified) column with exact [128,4] is_gt ops.
  * count (dly < 0.05) the same way via Sign(0.05 - dly).
  * count consecutive-size equality / direction flips with one fused
    VectorE tensor_tensor_reduce(is_equal / not_equal, accum_out=...) per
    tile, again patching the last column separately.
  * everything per-sample afterwards runs on tiny [128, 4] tiles.
"""

import numpy as np
from contextlib import ExitStack

import concourse.bass as bass
import concourse.bacc as bacc
import concourse.mybir as mybir
from concourse import tile
from concourse.bass_utils import run_bass_kernel_spmd

B, S = 4096, 2048
N_CORES = 8
BC = B // N_CORES          # 512 rows per core
P = 128                    # SBUF partitions
NT = BC // P               # 4 tiles of 128 rows per core

F32 = mybir.dt.float32
I32 = mybir.dt.int32
ALU = mybir.AluOpType
ACTF = mybir.ActivationFunctionType

_NC_CACHE = None
LAST_RESULTS = None        # BassKernelResults of the last kernel() call


def _build_nc() -> bass.Bass:
    nc = bacc.Bacc()

    sz_h = nc.declare_dram_parameter("raw_sizes", [BC, S], I32, isOutput=False)
    dl_h = nc.declare_dram_parameter("raw_delays", [BC, S], F32, isOutput=False)
    dr_h = nc.declare_dram_parameter("raw_directions", [BC, S], I32, isOutput=False)
    dms_h = nc.declare_dram_parameter("delay_ms", [BC], F32, isOutput=False)
    pad_h = nc.declare_dram_parameter("padding_norm", [BC], F32, isOutput=False)
    cnf_h = nc.declare_dram_parameter("confidence", [BC], F32, isOutput=False)
    pid_h = nc.declare_dram_parameter("profile_ids", [BC], I32, isOutput=False)
    out_h = nc.declare_dram_parameter("partial", [P, 1], F32, isOutput=True)

    with tile.TileContext(nc) as tc, ExitStack() as ctx:
        inp = ctx.enter_context(tc.tile_pool(name="inp", bufs=4))
        scr = ctx.enter_context(tc.tile_pool(name="scr", bufs=2))
        sm = ctx.enter_context(tc.tile_pool(name="sm", bufs=1))

        def smt(tag, dtype=F32):
            return sm.tile([P, NT], dtype, tag=tag, name=tag)

        _consts = {}

        def constv(val):
            """[128,1] f32 SBUF tile holding `val` (for activation bias APs)."""
            if val not in _consts:
                cname = f"cst{len(_consts)}"
                ct = sm.tile([P, 1], F32, tag=cname, name=cname)
                nc.vector.memset(ct[:, :], val)
                _consts[val] = ct[:, :]
            return _consts[val]

        # Row mapping: core row r -> (partition p, tile t) with r = p*NT + t.
        # This makes the per-row [128, NT] vector loads a dense 2D DMA
        # (partition stride 16B) instead of a 512-descriptor gather, while
        # the big tile loads just become row-strided (stride NT rows), which
        # costs the same descriptors as contiguous rows.
        dvec = smt("dvec")
        pvec = smt("pvec")
        cvec = smt("cvec")
        pidt = smt("pidt", I32)

        # Big-op accumulators (per tile column), split into column halves so
        # the last tile's compute tail after its DMA is only a half-op deep
        R1a, R1b = smt("R1a"), smt("R1b")   # sum sign(sz - 1400.5)
        R2a, R2b = smt("R2a"), smt("R2b")   # sum sign(0.05 - dly)
        R3a, R3b = smt("R3a"), smt("R3b")   # count sz[s] == sz[s-1], s=1..S-1
        R4a, R4b = smt("R4a"), smt("R4b")   # count dir[s] != dir[s-1], s=1..S-1
        szlast = smt("szlast")   # f32 copy of int sz[:, S-1]
        szprev = smt("szprev")   # f32 copy of int sz[:, S-2]
        dllast = smt("dllast")   # f32 copy of dly[:, S-1]

        sz_t = sz_h[:, :].rearrange("(p t) s -> t p s", t=NT)
        dl_t = dl_h[:, :].rearrange("(p t) s -> t p s", t=NT)
        dr_t = dr_h[:, :].rearrange("(p t) s -> t p s", t=NT)
        H = S // 2
        for t in range(NT):
            szt = inp.tile([P, S], I32, tag="szt")
            dlt = inp.tile([P, S], F32, tag="dlt")
            drt = inp.tile([P, S], I32, tag="drt")
            for h, cs in ((0, slice(0, H)), (1, slice(H, S))):
                nc.sync.dma_start(szt[:, cs], sz_t[t][:, cs])
                nc.sync.dma_start(dlt[:, cs], dl_t[t][:, cs])
                nc.sync.dma_start(drt[:, cs], dr_t[t][:, cs])

            col = slice(t, t + 1)
            o1 = scr.tile([P, S], F32, tag="o1")
            nc.scalar.activation(o1[:, 0:H], szt[:, 0:H], ACTF.Sign,
                                 bias=constv(-1400.5), scale=1.0, accum_out=R1a[:, col])
            nc.scalar.activation(o1[:, H:S], szt[:, H:S], ACTF.Sign,
                                 bias=constv(-1400.5), scale=1.0, accum_out=R1b[:, col])
            o2 = scr.tile([P, S], F32, tag="o2")
            nc.scalar.activation(o2[:, 0:H], dlt[:, 0:H], ACTF.Sign,
                                 bias=constv(0.05), scale=-1.0, accum_out=R2a[:, col])
            nc.scalar.activation(o2[:, H:S], dlt[:, H:S], ACTF.Sign,
                                 bias=constv(0.05), scale=-1.0, accum_out=R2b[:, col])
            # fused compare + row-sum on DVE: out = (in0 bypass 0) cmp in1,
            # accum_out = sum(out).  (tensor_tensor_reduce crashes the HW
            # runtime in this toolchain; scalar_tensor_tensor w/ accum works.)
            o3 = scr.tile([P, S - 1], F32, tag="o3")
            nc.vector.scalar_tensor_tensor(
                o3[:, 0:H - 1], szt[:, 1:H], 0.0, szt[:, 0:H - 1],
                ALU.bypass, ALU.is_equal, accum_out=R3a[:, col])
            nc.vector.scalar_tensor_tensor(
                o3[:, H - 1:S - 1], szt[:, H:S], 0.0, szt[:, H - 1:S - 1],
                ALU.bypass, ALU.is_equal, accum_out=R3b[:, col])
            o4 = scr.tile([P, S - 1], F32, tag="o4")
            nc.vector.scalar_tensor_tensor(
                o4[:, 0:H - 1], drt[:, 1:H], 0.0, drt[:, 0:H - 1],
                ALU.bypass, ALU.not_equal, accum_out=R4a[:, col])
            nc.vector.scalar_tensor_tensor(
                o4[:, H - 1:S - 1], drt[:, H:S], 0.0, drt[:, H - 1:S - 1],
                ALU.bypass, ALU.not_equal, accum_out=R4b[:, col])

            nc.vector.tensor_copy(szlast[:, col], szt[:, S - 1:S])
            nc.vector.tensor_copy(szprev[:, col], szt[:, S - 2:S - 1])
            nc.vector.tensor_copy(dllast[:, col], dlt[:, S - 1:S])

        # per-row vectors loaded after the big streams are queued (tiny DMAs)
        nc.gpsimd.dma_start(dvec[:, :], dms_h[:].rearrange("(p t) -> p t", t=NT))
        nc.gpsimd.dma_start(pvec[:, :], pad_h[:].rearrange("(p t) -> p t", t=NT))
        nc.gpsimd.dma_start(cvec[:, :], cnf_h[:].rearrange("(p t) -> p t", t=NT))
        nc.gpsimd.dma_start(pidt[:, :], pid_h[:].rearrange("(p t) -> p t", t=NT))

        # ---- per-sample combine, all on [128, 4] tiles (VectorE only,
        # to keep per-instruction sync-wait counts low on ScalarE) ----
        v = nc.vector

        # merge column-half accumulators
        R1, R2, R3, R4 = smt("R1"), smt("R2"), smt("R3"), smt("R4")
        v.tensor_add(R1[:, :], R1a[:, :], R1b[:, :])
        v.tensor_add(R2[:, :], R2a[:, :], R2b[:, :])
        v.tensor_add(R3[:, :], R3a[:, :], R3b[:, :])
        v.tensor_add(R4[:, :], R4a[:, :], R4b[:, :])

        # profile-id one-hots (pid in 0..4)
        pidf = smt("pidf")
        v.tensor_copy(pidf[:, :], pidt[:, :])
        e1, e2, e3, e4 = smt("e1"), smt("e2"), smt("e3"), smt("e4")
        v.tensor_scalar(e1[:, :], pidf[:, :], 1.0, None, ALU.is_equal)
        v.tensor_scalar(e2[:, :], pidf[:, :], 2.0, None, ALU.is_equal)
        v.tensor_scalar(e3[:, :], pidf[:, :], 3.0, None, ALU.is_equal)
        v.tensor_scalar(e4[:, :], pidf[:, :], 4.0, None, ALU.is_equal)

        # CONFIG_MULT[pid % 4] = 1.0 + 0.3*e1 + 0.6*e2 + 1.0*e3  (pid=4 -> 1.0)
        mlt = smt("mlt")
        v.tensor_scalar(mlt[:, :], e1[:, :], 0.3, 1.0, ALU.mult, ALU.add)
        v.scalar_tensor_tensor(mlt[:, :], e2[:, :], 0.6, mlt[:, :], ALU.mult, ALU.add)
        v.tensor_add(mlt[:, :], mlt[:, :], e3[:, :])

        # TARGET_DELAY[pid] = 2 - 1*e1 - 1.5*e2 + 3*e3 + 1*e4
        td = smt("td")
        v.tensor_scalar(td[:, :], e1[:, :], -1.0, 2.0, ALU.mult, ALU.add)
        v.scalar_tensor_tensor(td[:, :], e2[:, :], -1.5, td[:, :], ALU.mult, ALU.add)
        v.scalar_tensor_tensor(td[:, :], e3[:, :], 3.0, td[:, :], ALU.mult, ALU.add)
        v.tensor_add(td[:, :], td[:, :], e4[:, :])

        # TARGET_PAD[pid] = 0.08 + 0.04*e1 - 0.03*e2 + 0.07*e3 + 0.02*e4
        tp = smt("tp")
        v.tensor_scalar(tp[:, :], e1[:, :], 0.04, 0.08, ALU.mult, ALU.add)
        v.scalar_tensor_tensor(tp[:, :], e2[:, :], -0.03, tp[:, :], ALU.mult, ALU.add)
        v.scalar_tensor_tensor(tp[:, :], e3[:, :], 0.07, tp[:, :], ALU.mult, ALU.add)
        v.scalar_tensor_tensor(tp[:, :], e4[:, :], 0.02, tp[:, :], ALU.mult, ALU.add)

        # last-column morphing fixups
        padx = smt("padx")
        v.tensor_scalar(padx[:, :], pvec[:, :], 1500.0, None, ALU.mult)
        szmod = smt("szmod")
        v.tensor_add(szmod[:, :], szlast[:, :], padx[:, :])
        v.tensor_scalar(szmod[:, :], szmod[:, :], 1500.0, None, ALU.min)
        dlmod = smt("dlmod")
        v.tensor_add(dlmod[:, :], dllast[:, :], dvec[:, :])

        g1m, g1r = smt("g1m"), smt("g1r")
        v.tensor_scalar(g1m[:, :], szmod[:, :], 1400.0, None, ALU.is_gt)
        v.tensor_scalar(g1r[:, :], szlast[:, :], 1400.0, None, ALU.is_gt)
        l2m, l2r = smt("l2m"), smt("l2r")
        v.tensor_scalar(l2m[:, :], dlmod[:, :], 0.05, None, ALU.is_lt)
        v.tensor_scalar(l2r[:, :], dllast[:, :], 0.05, None, ALU.is_lt)
        e3r = smt("e3r")
        v.tensor_tensor(e3r[:, :], szlast[:, :], szprev[:, :], ALU.is_equal)
        d3 = smt("d3")
        v.tensor_sub(d3[:, :], szmod[:, :], szprev[:, :])
        a3 = smt("a3")
        nc.scalar.activation(a3[:, :], d3[:, :], ACTF.Abs)
        e3m = smt("e3m")
        v.tensor_scalar(e3m[:, :], a3[:, :], 0.5, None, ALU.is_lt)

        # exact per-row counts
        cnt1 = smt("cnt1")
        v.tensor_scalar(cnt1[:, :], R1[:, :], 0.5, float(S) / 2, ALU.mult, ALU.add)
        v.tensor_sub(cnt1[:, :], cnt1[:, :], g1r[:, :])
        v.tensor_add(cnt1[:, :], cnt1[:, :], g1m[:, :])
        cnt2 = smt("cnt2")
        v.tensor_scalar(cnt2[:, :], R2[:, :], 0.5, float(S) / 2, ALU.mult, ALU.add)
        v.tensor_sub(cnt2[:, :], cnt2[:, :], l2r[:, :])
        v.tensor_add(cnt2[:, :], cnt2[:, :], l2m[:, :])
        cnt3 = smt("cnt3")
        v.tensor_sub(cnt3[:, :], R3[:, :], e3r[:, :])
        v.tensor_add(cnt3[:, :], cnt3[:, :], e3m[:, :])

        # scores = (0.6*c1 + 0.4*c2 + 0.2*c3 + 0.1*c4 + 0.1) * (100/S) * mult
        acc = smt("acc")
        v.tensor_scalar(acc[:, :], cnt1[:, :], 0.6, None, ALU.mult)
        v.scalar_tensor_tensor(acc[:, :], cnt2[:, :], 0.4, acc[:, :], ALU.mult, ALU.add)
        v.scalar_tensor_tensor(acc[:, :], cnt3[:, :], 0.2, acc[:, :], ALU.mult, ALU.add)
        v.scalar_tensor_tensor(acc[:, :], R4[:, :], 0.1, acc[:, :], ALU.mult, ALU.add)
        base = smt("base")
        v.tensor_scalar(base[:, :], acc[:, :], 100.0 / S, 0.1 * 100.0 / S,
                        ALU.mult, ALU.add)
        scores = smt("scores")
        v.tensor_mul(scores[:, :], base[:, :], mlt[:, :])

        ev = smt("ev")
        v.tensor_scalar(ev[:, :], scores[:, :], 30.0, None, ALU.is_lt)
        dpi = smt("dpi")
        v.tensor_scalar(dpi[:, :], scores[:, :], 15.0, -15.0, ALU.max, ALU.add)

        sd = smt("sd")
        v.tensor_sub(sd[:, :], dvec[:, :], td[:, :])
        sda = smt("sda")
        nc.scalar.activation(sda[:, :], sd[:, :], ACTF.Abs)
        sp = smt("sp")
        v.tensor_sub(sp[:, :], pvec[:, :], tp[:, :])
        spa = smt("spa")
        nc.scalar.activation(spa[:, :], sp[:, :], ACTF.Abs)
        sim = smt("sim")
        v.tensor_add(sim[:, :], sda[:, :], spa[:, :])

        ed = smt("ed")
        v.tensor_scalar(ed[:, :], dvec[:, :], 20.0, -20.0, ALU.max, ALU.add)
        ep = smt("ep")
        v.tensor_scalar(ep[:, :], pvec[:, :], 0.3, -0.3, ALU.max, ALU.add)
        eff = smt("eff")
        v.scalar_tensor_tensor(eff[:, :], ed[:, :], 1.0 / 20.0, ep[:, :],
                               ALU.mult, ALU.add)

        cd = smt("cd")
        v.tensor_sub(cd[:, :], cvec[:, :], ev[:, :])
        cq = smt("cq")
        v.tensor_mul(cq[:, :], cd[:, :], cd[:, :])

        ctot = smt("ctot")
        v.tensor_scalar(ctot[:, :], dpi[:, :], 2.0 / 30.0, None, ALU.mult)
        v.scalar_tensor_tensor(ctot[:, :], sim[:, :], 0.5, ctot[:, :], ALU.mult, ALU.add)
        v.scalar_tensor_tensor(ctot[:, :], eff[:, :], 0.3, ctot[:, :], ALU.mult, ALU.add)
        v.scalar_tensor_tensor(ctot[:, :], cq[:, :], 0.2, ctot[:, :], ALU.mult, ALU.add)

        red = sm.tile([P, 1], F32, tag="red", name="red")
        v.tensor_reduce(red[:, :], ctot[:, :], axis=mybir.AxisListType.X, op=ALU.add)
        nc.sync.dma_start(out_h[:, :], red[:, :])

    nc.finalize()
    return nc


def _get_nc() -> bass.Bass:
    global _NC_CACHE
    if _NC_CACHE is None:
        _NC_CACHE = _build_nc()
    return _NC_CACHE


def kernel(raw_sizes, raw_delays, raw_directions, delay_ms, padding_norm,
           confidence, profile_ids, trace=False, tmpdir=None):
    global LAST_RESULTS
    raw_sizes = np.asarray(raw_sizes, dtype=np.int32)
    raw_delays = np.asarray(raw_delays, dtype=np.float32)
    raw_directions = np.asarray(raw_directions, dtype=np.int32)
    delay_ms = np.asarray(delay_ms, dtype=np.float32)
    padding_norm = np.asarray(padding_norm, dtype=np.float32)
    confidence = np.asarray(confidence, dtype=np.float32)
    profile_ids = np.asarray(profile_ids).astype(np.int32)

    nc = _get_nc()
    in_maps = []
    for i in range(N_CORES):
        r = slice(i * BC, (i + 1) * BC)
        in_maps.append({
            "raw_sizes": raw_sizes[r],
            "raw_delays": raw_delays[r],
            "raw_directions": raw_directions[r],
            "delay_ms": delay_ms[r],
            "padding_norm": padding_norm[r],
            "confidence": confidence[r],
            "profile_ids": profile_ids[r],
        })

    LAST_RESULTS = run_bass_kernel_spmd(nc, in_maps, list(range(N_CORES)),
                                        trace=trace, tmpdir=tmpdir)
    partials = [LAST_RESULTS.results[i]["partial"] for i in range(N_CORES)]
    total = float(np.sum(np.stack(partials), dtype=np.float64))
    return np.float32(total / B)
